# revision 34
# baseline (speedup 1.0000x reference)
"""Distributed Trainium2 (8 NeuronCores) attention-head kernel, v2.

Problem: single attention head with projections.
  q = Q @ Wq.T + bq ; k = K @ Wk.T + bk ; v = V @ Wv.T + bv
  x = (q @ k.T) / 8 ; x = x*m - 1e9*(1-m) ; p = softmax(x) ; y = p @ v
Shapes: Q/K/V [2, 4096, 1024] f32, mask [2, 4096, 4096] int32 -> y [2, 4096, 64].

Sharding (8 cores): 2x2 grid per batch (flash-decoding style per the hint):
core (b, qh, kh) handles 2048 queries x 2048 keys and returns UNNORMALIZED
partial stats yT[65, 2048] = [sum_s p_s v_s ; sum_s p_s]; the host combines
the two kh partials per (b, qh): y = (yA+yB)[:64] / (yA+yB)[64].  This is the
"all-gathered softmax statistics" combine done at unshard time (collectives
on this fleet cost ~100us fixed, host combine is ~2M flops).

Device pipeline (all matmuls bf16, psum f32):
  - projections col-tiled (out width 64 -> two 64-row col strips run
    concurrently in the PE array); qT is produced duplicated on both
    partition halves, kT split even/odd chunk so scores can row-tile.
  - scores: contraction is only dk=64, so 4 (K=64, M=64) tiles run
    concurrently via tile_position row+col strips (~2x).
  - mask: folded into the scores PSUM by an fp8 DoubleRow identity matmul
    (psum += 240*m), then ACT computes p = exp(0.125*s + 30m - 30) in one
    pass - the masked softmax exactly (leak exp(-30+6) ~ 4e-11, negligible).
    No DVE/Pool elementwise mask work, mask DMA stays 1 byte/elem.
  - y: yT[65, :] += v_aug.T @ p accumulated over key chunks (v_aug has a
    ones column -> row 64 = sum p).
  - PE warmup matmuls at t=0 engage the HAM clock gate (1.2 -> 2.4 GHz).
"""

import numpy as np
import ml_dtypes

import concourse.bass as bass
import concourse.mybir as mybir
import concourse.tile as tile
from concourse import bacc
from concourse.bass_utils import run_bass_kernel_spmd
from concourse.masks import make_identity

B, S, DM, DK = 2, 4096, 1024, 64
N_CORES = 8
SQ = 2048            # queries per core
SK = 2048            # keys per core
NG = 8               # key groups per core (256 keys each)
NJ = DM // 128       # dm chunks (8)

F32 = mybir.dt.float32
BF16 = mybir.dt.bfloat16
FP8 = mybir.dt.float8e4
DR = mybir.MatmulPerfMode.DoubleRow
EXP = mybir.ActivationFunctionType.Exp

MASK_W = 240.0       # ident weight: exp(0.125*(s + 240*m) - 30) = exp(s/8 + 30m - 30)
N_WARM = 14          # narrow PE warmup matmuls: span the HAM window until qt lands
DEBUG = False        # add intermediate dumps

_last_results = None


def _build():
    nc = bacc.Bacc(None, target_bir_lowering=False)

    qt_e = nc.declare_dram_parameter("qt", [128, 2 * NJ * 1024], BF16, isOutput=False)
    kt_e = nc.declare_dram_parameter("kt", [128, NG * NJ * 256], BF16, isOutput=False)
    vt_e = nc.declare_dram_parameter("vt", [128, NG * NJ * 256], BF16, isOutput=False)
    m0_e = nc.declare_dram_parameter("m0", [128, 16, 1024], FP8, isOutput=False)
    m1_e = nc.declare_dram_parameter("m1", [128, 16, 1024], FP8, isOutput=False)
    w_e = nc.declare_dram_parameter("wqkv", [128, 3 * NJ * DK], BF16, isOutput=False)
    b_e = nc.declare_dram_parameter("bqkv", [128, 3], F32, isOutput=False)
    id_e = nc.declare_dram_parameter("identdr", [128, 128], FP8, isOutput=False)
    out_e = nc.declare_dram_parameter("out", [65, SQ], F32, isOutput=True)
    if DEBUG:
        dbg_e = {
            "d_qT0": nc.declare_dram_parameter("d_qT0", [128, 1024], BF16, isOutput=True),
            "d_kT": nc.declare_dram_parameter("d_kT", [128, NG * 128], BF16, isOutput=True),
            "d_vaug": nc.declare_dram_parameter("d_vaug", [128, 16 * 65], BF16, isOutput=True),
            "d_p": nc.declare_dram_parameter("d_p", [128, 1024], BF16, isOutput=True),
        }

    with tile.TileContext(nc) as tc:
        with (
            tc.tile_pool(name="const", bufs=1) as cpool,
            tc.tile_pool(name="inp", bufs=1) as ipool,
            tc.tile_pool(name="work", bufs=1) as spool,
            tc.tile_pool(name="pp", bufs=3) as ppool,
            tc.tile_pool(name="ps_work", bufs=2, space="PSUM") as pwork,
            tc.tile_pool(name="ps_y", bufs=1, space="PSUM") as py,
            tc.tile_pool(name="ps_kv", bufs=1, space="PSUM") as pkv,
        ):
            # ---- constants / warmup (no DMA deps) ----
            wu = cpool.tile([128, 512], BF16, tag="wu")
            nc.vector.memset(wu[:], 0.0)
            nbias = cpool.tile([128, 1], F32, tag="nbias")
            nc.vector.memset(nbias[:], -30.0)
            act_w = spool.tile([128, 32], BF16, tag="actw")
            nc.scalar.activation(act_w[:], wu[:, 0:32], EXP, bias=nbias[:])  # pull exp tables early
            ident_bf = cpool.tile([128, 128], BF16, tag="identbf")
            make_identity(nc, ident_bf[:])

            wups = pwork.tile([128, 1024], F32, tag="sAB", name="wups")
            for i in range(N_WARM):
                nc.tensor.matmul(
                    wups[:, 0:128], lhsT=wu[:, 0:128], rhs=wu[:, 0:128],
                    start=True, stop=True, skip_group_check=True,
                )

            # ---- input DMAs (issue order ~= arrival order) ----
            qt_sb = ipool.tile([128, 2 * NJ * 1024], BF16, tag="qt")
            nc.sync.dma_start(qt_sb[:, 0:4096], qt_e[:, 0:4096])
            nc.sync.dma_start(qt_sb[:, 4096:8192], qt_e[:, 4096:8192])
            w_sb = cpool.tile([128, 3 * NJ * DK], BF16, tag="w")
            nc.sync.dma_start(w_sb[:], w_e[:])
            b_sb = cpool.tile([128, 3], F32, tag="b")
            nc.sync.dma_start(b_sb[:], b_e[:])
            id_sb = cpool.tile([128, 128], FP8, tag="ident")
            nc.sync.dma_start(id_sb[:], id_e[:])

            kt_sb = ipool.tile([128, NG * 2048], BF16, tag="kt")
            vt_sb = ipool.tile([128, NG * 2048], BF16, tag="vt")
            m0_sb = ipool.tile([128, 16, 1024], FP8, tag="m0")
            m1_sb = ipool.tile([128, 16, 1024], FP8, tag="m1")
            for g in range(NG):   # per-group blocks: smoother pass-0 gating
                cs = slice(g * 2048, (g + 1) * 2048)
                # split dispatch across the two HWDGE queues (Sync + Activation,
                # which is idle until the first exp) to feed DMA engines earlier
                nc.sync.dma_start(kt_sb[:, cs], kt_e[:, cs])
                nc.scalar.dma_start(vt_sb[:, cs], vt_e[:, cs])
                nc.scalar.dma_start(m0_sb[:, 2 * g:2 * g + 2, :], m0_e[:, 2 * g:2 * g + 2, :])
                if g == 1:
                    nc.sync.dma_start(qt_sb[:, 8192:16384], qt_e[:, 8192:16384])
            nc.sync.dma_start(m1_sb[:], m1_e[:])

            def wsl(which, j):
                return w_sb[:, (which * NJ + j) * DK:(which * NJ + j + 1) * DK]

            # ---- persistent work tiles ----
            qT = {}
            kT = spool.tile([128, NG * 128], BF16, tag="kT")
            vT = spool.tile([128, NG * 128], BF16, tag="vT")
            v_aug = spool.tile([128, 16 * 65], BF16, tag="vaug")
            nc.vector.memset(v_aug[:], 1.0)

            def q_proj(h):
                qps = pwork.tile([128, 1024], F32, tag="sAB", name=f"qps{h}")
                for j in range(NJ):
                    for s in range(2):
                        rhs = qt_sb[:, h * 8192 + j * 1024 + s * 512:
                                    h * 8192 + j * 1024 + (s + 1) * 512]
                        for st in range(2):
                            nc.tensor.matmul(
                                qps[st * 64:(st + 1) * 64, s * 512:(s + 1) * 512],
                                lhsT=wsl(0, j), rhs=rhs,
                                start=(j == 0), stop=(j == NJ - 1),
                            )
                qT[h] = spool.tile([128, 1024], BF16, tag=f"qT{h}", name=f"qT{h}")
                nc.vector.tensor_scalar_add(qT[h][:], qps[:], b_sb[:, 0:1])

            def kv_proj(g):
                kps = pkv.tile([128, 128], F32, tag="kps", name=f"kps{g}")
                for j in range(NJ):
                    c0 = g * 2048 + j * 256
                    nc.tensor.matmul(
                        kps[0:64, :], lhsT=wsl(1, j), rhs=kt_sb[:, c0:c0 + 128],
                        start=(j == 0), stop=(j == NJ - 1),
                    )
                    nc.tensor.matmul(
                        kps[64:128, :], lhsT=wsl(1, j), rhs=kt_sb[:, c0 + 128:c0 + 256],
                        start=(j == 0), stop=(j == NJ - 1),
                    )
                nc.vector.tensor_scalar_add(
                    kT[:, g * 128:(g + 1) * 128], kps[:], b_sb[:, 1:2]
                )
                vps = pkv.tile([128, 128], F32, tag="vps", name=f"vps{g}")
                for j in range(NJ):
                    c0 = g * 2048 + j * 256
                    nc.tensor.matmul(
                        vps[0:64, :], lhsT=wsl(2, j), rhs=vt_sb[:, c0:c0 + 128],
                        start=(j == 0), stop=(j == NJ - 1),
                    )
                    nc.tensor.matmul(
                        vps[64:128, :], lhsT=wsl(2, j), rhs=vt_sb[:, c0 + 128:c0 + 256],
                        start=(j == 0), stop=(j == NJ - 1),
                    )
                nc.vector.tensor_scalar_add(
                    vT[:, g * 128:(g + 1) * 128], vps[:], b_sb[:, 2:3]
                )
                for c in range(2):
                    vtr = pkv.tile([128, 64], BF16, tag="kps", name=f"vtr{g}_{c}")
                    nc.tensor.transpose(
                        vtr[:], vT[c * 64:(c + 1) * 64, g * 128:(g + 1) * 128],
                        ident_bf[c * 64:(c + 1) * 64, c * 64:(c + 1) * 64],
                    )
                    nc.vector.tensor_copy(
                        v_aug[:, (2 * g + c) * 65:(2 * g + c) * 65 + 64], vtr[:]
                    )

            def main_step(g, h, s, y_ps, m_sb):
                """Emit scores+mask+ACT for (g, s); return a closure emitting the
                y matmuls (deferred one step so the in-order PE never waits on ACT)."""
                sAB = pwork.tile([128, 1024], F32, tag="sAB", name=f"s{h}_{g}_{s}")
                qc = slice(s * 512, (s + 1) * 512)
                kc = g * 128
                # scores: 4 concurrent (K=64, M=64) tiles
                nc.tensor.matmul(
                    sAB[0:64, 0:512], lhsT=kT[0:64, kc:kc + 64],
                    rhs=qT[h][0:64, qc], start=True, stop=False,
                    skip_group_check=True,
                )
                nc.tensor.matmul(
                    sAB[64:128, 0:512], lhsT=kT[0:64, kc + 64:kc + 128],
                    rhs=qT[h][0:64, qc], start=True, stop=False,
                    skip_group_check=True,
                )
                nc.tensor.matmul(
                    sAB[0:64, 512:1024], lhsT=kT[64:128, kc:kc + 64],
                    rhs=qT[h][64:128, qc], start=True, stop=False,
                    skip_group_check=True,
                )
                nc.tensor.matmul(
                    sAB[64:128, 512:1024], lhsT=kT[64:128, kc + 64:kc + 128],
                    rhs=qT[h][64:128, qc], start=True, stop=False,
                    skip_group_check=True,
                )
                # mask add: psum += 240*m via plain fp8 identity (FWL weights)
                nc.tensor.matmul(
                    sAB[:, 0:512], lhsT=id_sb[:],
                    rhs=m_sb[:, 2 * g:2 * g + 1, s * 512:(s + 1) * 512],
                    start=False, stop=True, skip_group_check=True,
                )
                nc.tensor.matmul(
                    sAB[:, 512:1024], lhsT=id_sb[:],
                    rhs=m_sb[:, 2 * g + 1:2 * g + 2, s * 512:(s + 1) * 512],
                    start=False, stop=True, skip_group_check=True,
                )
                p = ppool.tile([128, 1024], BF16, tag="p", name=f"p{h}_{g}_{s}")
                nc.scalar.activation(p[:], sAB[:], EXP, bias=nbias[:], scale=0.125)
                if DEBUG and (g, h, s) == (0, 0, 0):
                    nc.sync.dma_start(dbg_e["d_p"][:], p[:])

                def emit_y():
                    nc.tensor.matmul(
                        y_ps[:, qc], lhsT=v_aug[:, (2 * g) * 65:(2 * g) * 65 + 65],
                        rhs=p[:, 0:512], start=(g == 0), stop=False,
                        skip_group_check=True,
                    )
                    nc.tensor.matmul(
                        y_ps[:, qc], lhsT=v_aug[:, (2 * g + 1) * 65:(2 * g + 1) * 65 + 65],
                        rhs=p[:, 512:1024], start=False, stop=(g == NG - 1),
                        skip_group_check=True,
                    )
                return emit_y

            # ---- pass 0 (q half 0) with per-group projections ----
            with nc.named_scope("qproj0"):
                q_proj(0)
            y0 = py.tile([65, 1024], F32, tag="y", name="y0")
            pend = None
            for g in range(NG):
                with nc.named_scope(f"kv{g}"):
                    kv_proj(g)
                with nc.named_scope(f"p0g{g}"):
                    for s in range(2):
                        ey = main_step(g, 0, s, y0, m0_sb)
                        if pend is not None:
                            pend()
                        pend = ey
                if g == 2:
                    # qT for half 1 while pass 0 is DMA-gated
                    with nc.named_scope("qproj1"):
                        q_proj(1)
            pend()
            ysb0 = spool.tile([65, 1024], F32, tag="ysb0")
            nc.vector.tensor_copy(ysb0[:], y0[:])
            nc.sync.dma_start(out_e[:, 0:512], ysb0[:, 0:512])
            nc.sync.dma_start(out_e[:, 512:1024], ysb0[:, 512:1024])

            # ---- pass 1 (q half 1) ----
            y1 = py.tile([65, 1024], F32, tag="y", name="y1")
            ysb1 = spool.tile([65, 1024], F32, tag="ysb1")
            pend = None
            for g in range(NG):
                with nc.named_scope(f"p1g{g}"):
                    for s in range(2):
                        ey = main_step(g, 1, s, y1, m1_sb)
                        if pend is not None:
                            pend()
                        if (g, s) == (NG - 1, 1):
                            # y region s=0 is complete: drain it now
                            nc.vector.tensor_copy(ysb1[:, 0:512], y1[:, 0:512])
                            nc.sync.dma_start(out_e[:, 1024:1536], ysb1[:, 0:512])
                        pend = ey
            pend()
            nc.vector.tensor_copy(ysb1[:, 512:1024], y1[:, 512:1024])
            nc.sync.dma_start(out_e[:, 1536:2048], ysb1[:, 512:1024])

            if DEBUG:
                nc.sync.dma_start(dbg_e["d_qT0"][:], qT[0][:])
                nc.sync.dma_start(dbg_e["d_kT"][:], kT[:])
                nc.sync.dma_start(dbg_e["d_vaug"][:], v_aug[:])

    nc.finalize()
    return nc


def _pack_x(x):
    """[2048 rows, 1024 dm] f32 -> qt layout [128, 2*8*1024] (h, j, q')."""
    t = x.T.reshape(NJ, 128, 2, 1024)          # [j, p, h, q']
    return np.ascontiguousarray(
        t.transpose(1, 2, 0, 3).reshape(128, -1)
    ).astype(ml_dtypes.bfloat16)


def _pack_kv(x):
    """[2048 keys, 1024 dm] f32 -> [128, 8*8*256] (g, j, r)."""
    t = x.T.reshape(NJ, 128, NG, 256)          # [j, p, g, r]
    return np.ascontiguousarray(
        t.transpose(1, 2, 0, 3).reshape(128, -1)
    ).astype(ml_dtypes.bfloat16)


def _pack_mask(mblk):
    """mask block [2048 q, 2048 k] int -> (m0, m1) each [128, 16, 1024] fp8.
    element (key = g*256 + j*128 + p, q = h*1024 + q') at m{h}[p, 2g+j, q']."""
    t = mblk.T.reshape(NG, 2, 128, 2, 1024)    # [g, j, p, h, q']
    t = t.transpose(2, 3, 0, 1, 4)             # [p, h, g, j, q']
    m = np.ascontiguousarray(t.reshape(128, 2, 16, 1024)).astype(ml_dtypes.float8_e4m3)
    return m[:, 0], m[:, 1]


def kernel(Q, K, V, mask, Wq, bq, Wk, bk, Wv, bv):
    global _last_results
    bf16 = ml_dtypes.bfloat16
    fp8 = ml_dtypes.float8_e4m3

    Q, K, V = (np.asarray(a, dtype=np.float32) for a in (Q, K, V))
    mask = np.asarray(mask)

    w_p = np.concatenate(
        [np.ascontiguousarray(
            W.T.reshape(NJ, 128, DK).transpose(1, 0, 2).reshape(128, NJ * DK)
         ).astype(bf16) for W in (Wq, Wk, Wv)],
        axis=1,
    )
    b_p = np.ascontiguousarray(
        np.stack([np.tile(np.asarray(b, np.float32), 2) for b in (bq, bk, bv)], axis=1)
    )
    ident = (MASK_W * np.eye(128, dtype=np.float32)).astype(fp8)

    qt_c = {(b, qh): _pack_x(Q[b, qh * SQ:(qh + 1) * SQ]) for b in range(B) for qh in range(2)}
    kt_c = {(b, kh): _pack_kv(K[b, kh * SK:(kh + 1) * SK]) for b in range(B) for kh in range(2)}
    vt_c = {(b, kh): _pack_kv(V[b, kh * SK:(kh + 1) * SK]) for b in range(B) for kh in range(2)}

    in_maps = []
    for c in range(N_CORES):
        b, r = divmod(c, 4)
        qh, kh = divmod(r, 2)
        m0, m1 = _pack_mask(mask[b, qh * SQ:(qh + 1) * SQ, kh * SK:(kh + 1) * SK])
        in_maps.append({
            "qt": qt_c[(b, qh)], "kt": kt_c[(b, kh)], "vt": vt_c[(b, kh)],
            "m0": m0, "m1": m1,
            "wqkv": w_p, "bqkv": b_p, "identdr": ident,
        })

    nc = _build()
    res = run_bass_kernel_spmd(nc, in_maps, core_ids=list(range(N_CORES)))
    _last_results = res

    out = np.empty((B, S, DK), dtype=np.float32)
    for b in range(B):
        for qh in range(2):
            yA = res.results[b * 4 + qh * 2 + 0]["out"].astype(np.float64)
            yB = res.results[b * 4 + qh * 2 + 1]["out"].astype(np.float64)
            ysum = yA + yB
            y = ysum[:DK] / ysum[DK:DK + 1]
            out[b, qh * SQ:(qh + 1) * SQ, :] = y.T.astype(np.float32)
    return out


# revision 35
# speedup vs baseline: 1.0057x; 1.0057x over previous
"""Distributed Trainium2 (8 NeuronCores) attention-head kernel, v2.

Problem: single attention head with projections.
  q = Q @ Wq.T + bq ; k = K @ Wk.T + bk ; v = V @ Wv.T + bv
  x = (q @ k.T) / 8 ; x = x*m - 1e9*(1-m) ; p = softmax(x) ; y = p @ v
Shapes: Q/K/V [2, 4096, 1024] f32, mask [2, 4096, 4096] int32 -> y [2, 4096, 64].

Sharding (8 cores): 2x2 grid per batch (flash-decoding style per the hint):
core (b, qh, kh) handles 2048 queries x 2048 keys and returns UNNORMALIZED
partial stats yT[65, 2048] = [sum_s p_s v_s ; sum_s p_s]; the host combines
the two kh partials per (b, qh): y = (yA+yB)[:64] / (yA+yB)[64].  This is the
"all-gathered softmax statistics" combine done at unshard time (collectives
on this fleet cost ~100us fixed, host combine is ~2M flops).

Device pipeline (all matmuls bf16, psum f32):
  - projections col-tiled (out width 64 -> two 64-row col strips run
    concurrently in the PE array); qT is produced duplicated on both
    partition halves, kT split even/odd chunk so scores can row-tile.
  - scores: contraction is only dk=64, so 4 (K=64, M=64) tiles run
    concurrently via tile_position row+col strips (~2x).
  - mask: folded into the scores PSUM by an fp8 DoubleRow identity matmul
    (psum += 240*m), then ACT computes p = exp(0.125*s + 30m - 30) in one
    pass - the masked softmax exactly (leak exp(-30+6) ~ 4e-11, negligible).
    No DVE/Pool elementwise mask work, mask DMA stays 1 byte/elem.
  - y: yT[65, :] += v_aug.T @ p accumulated over key chunks (v_aug has a
    ones column -> row 64 = sum p).
  - PE warmup matmuls at t=0 engage the HAM clock gate (1.2 -> 2.4 GHz).
"""

import numpy as np
import ml_dtypes

import concourse.bass as bass
import concourse.mybir as mybir
import concourse.tile as tile
from concourse import bacc
from concourse.bass_utils import run_bass_kernel_spmd
from concourse.masks import make_identity

B, S, DM, DK = 2, 4096, 1024, 64
N_CORES = 8
SQ = 2048            # queries per core
SK = 2048            # keys per core
NG = 8               # key groups per core (256 keys each)
NJ = DM // 128       # dm chunks (8)

F32 = mybir.dt.float32
BF16 = mybir.dt.bfloat16
FP8 = mybir.dt.float8e4
DR = mybir.MatmulPerfMode.DoubleRow
EXP = mybir.ActivationFunctionType.Exp

MASK_W = 240.0       # ident weight: exp(0.125*(s + 240*m) - 30) = exp(s/8 + 30m - 30)
N_WARM = 14          # narrow PE warmup matmuls: span the HAM window until qt lands
DEBUG = False        # add intermediate dumps

_last_results = None


def _build():
    nc = bacc.Bacc(None, target_bir_lowering=False)

    qt_e = nc.declare_dram_parameter("qt", [128, 2 * NJ * 1024], BF16, isOutput=False)
    kt_e = nc.declare_dram_parameter("kt", [128, NG * NJ * 256], BF16, isOutput=False)
    vt_e = nc.declare_dram_parameter("vt", [128, NG * NJ * 256], BF16, isOutput=False)
    m0_e = nc.declare_dram_parameter("m0", [128, 16, 1024], FP8, isOutput=False)
    m1_e = nc.declare_dram_parameter("m1", [128, 16, 1024], FP8, isOutput=False)
    w_e = nc.declare_dram_parameter("wqkv", [128, 3 * NJ * DK], BF16, isOutput=False)
    b_e = nc.declare_dram_parameter("bqkv", [128, 3], F32, isOutput=False)
    id_e = nc.declare_dram_parameter("identdr", [128, 128], FP8, isOutput=False)
    out_e = nc.declare_dram_parameter("out", [65, SQ], F32, isOutput=True)
    if DEBUG:
        dbg_e = {
            "d_qT0": nc.declare_dram_parameter("d_qT0", [128, 1024], BF16, isOutput=True),
            "d_kT": nc.declare_dram_parameter("d_kT", [128, NG * 128], BF16, isOutput=True),
            "d_vaug": nc.declare_dram_parameter("d_vaug", [128, 16 * 65], BF16, isOutput=True),
            "d_p": nc.declare_dram_parameter("d_p", [128, 1024], BF16, isOutput=True),
        }

    with tile.TileContext(nc) as tc:
        with (
            tc.tile_pool(name="const", bufs=1) as cpool,
            tc.tile_pool(name="inp", bufs=1) as ipool,
            tc.tile_pool(name="work", bufs=1) as spool,
            tc.tile_pool(name="pp", bufs=3) as ppool,
            tc.tile_pool(name="ps_work", bufs=2, space="PSUM") as pwork,
            tc.tile_pool(name="ps_y", bufs=1, space="PSUM") as py,
            tc.tile_pool(name="ps_kv", bufs=1, space="PSUM") as pkv,
        ):
            # ---- constants / warmup (no DMA deps) ----
            wu = cpool.tile([128, 512], BF16, tag="wu")
            nc.vector.memset(wu[:], 0.0)
            nbias = cpool.tile([128, 1], F32, tag="nbias")
            nc.vector.memset(nbias[:], -30.0)
            act_w = spool.tile([128, 32], BF16, tag="actw")
            nc.scalar.activation(act_w[:], wu[:, 0:32], EXP, bias=nbias[:])  # pull exp tables early
            ident_bf = cpool.tile([128, 128], BF16, tag="identbf")
            make_identity(nc, ident_bf[:])

            wups = pwork.tile([128, 1024], F32, tag="sAB", name="wups")
            for i in range(N_WARM):
                nc.tensor.matmul(
                    wups[:, 0:128], lhsT=wu[:, 0:128], rhs=wu[:, 0:128],
                    start=True, stop=True, skip_group_check=True,
                )

            # ---- input DMAs (issue order ~= arrival order) ----
            qt_sb = ipool.tile([128, 2 * NJ * 1024], BF16, tag="qt")
            nc.sync.dma_start(qt_sb[:, 0:4096], qt_e[:, 0:4096])
            nc.sync.dma_start(qt_sb[:, 4096:8192], qt_e[:, 4096:8192])
            w_sb = cpool.tile([128, 3 * NJ * DK], BF16, tag="w")
            nc.sync.dma_start(w_sb[:], w_e[:])
            b_sb = cpool.tile([128, 3], F32, tag="b")
            nc.sync.dma_start(b_sb[:], b_e[:])
            id_sb = cpool.tile([128, 128], FP8, tag="ident")
            nc.sync.dma_start(id_sb[:], id_e[:])

            kt_sb = ipool.tile([128, NG * 2048], BF16, tag="kt")
            vt_sb = ipool.tile([128, NG * 2048], BF16, tag="vt")
            m0_sb = ipool.tile([128, 16, 1024], FP8, tag="m0")
            m1_sb = ipool.tile([128, 16, 1024], FP8, tag="m1")
            for g in range(NG):   # per-group blocks: smoother pass-0 gating
                cs = slice(g * 2048, (g + 1) * 2048)
                nc.sync.dma_start(kt_sb[:, cs], kt_e[:, cs])
                nc.sync.dma_start(vt_sb[:, cs], vt_e[:, cs])
                nc.sync.dma_start(m0_sb[:, 2 * g:2 * g + 2, :], m0_e[:, 2 * g:2 * g + 2, :])
                if g == 1:
                    nc.sync.dma_start(qt_sb[:, 8192:16384], qt_e[:, 8192:16384])
            nc.sync.dma_start(m1_sb[:], m1_e[:])

            def wsl(which, j):
                return w_sb[:, (which * NJ + j) * DK:(which * NJ + j + 1) * DK]

            # ---- persistent work tiles ----
            qT = {}
            kT = spool.tile([128, NG * 128], BF16, tag="kT")
            vT = spool.tile([128, NG * 128], BF16, tag="vT")
            v_aug = spool.tile([128, 16 * 65], BF16, tag="vaug")
            nc.vector.memset(v_aug[:], 1.0)

            def q_proj(h):
                qps = pwork.tile([128, 1024], F32, tag="sAB", name=f"qps{h}")
                for j in range(NJ):
                    for s in range(2):
                        rhs = qt_sb[:, h * 8192 + j * 1024 + s * 512:
                                    h * 8192 + j * 1024 + (s + 1) * 512]
                        for st in range(2):
                            nc.tensor.matmul(
                                qps[st * 64:(st + 1) * 64, s * 512:(s + 1) * 512],
                                lhsT=wsl(0, j), rhs=rhs,
                                start=(j == 0), stop=(j == NJ - 1),
                            )
                qT[h] = spool.tile([128, 1024], BF16, tag=f"qT{h}", name=f"qT{h}")
                nc.vector.tensor_scalar_add(qT[h][:], qps[:], b_sb[:, 0:1])

            def kv_proj(g):
                kps = pkv.tile([128, 128], F32, tag="kps", name=f"kps{g}")
                for j in range(NJ):
                    c0 = g * 2048 + j * 256
                    nc.tensor.matmul(
                        kps[0:64, :], lhsT=wsl(1, j), rhs=kt_sb[:, c0:c0 + 128],
                        start=(j == 0), stop=(j == NJ - 1),
                    )
                    nc.tensor.matmul(
                        kps[64:128, :], lhsT=wsl(1, j), rhs=kt_sb[:, c0 + 128:c0 + 256],
                        start=(j == 0), stop=(j == NJ - 1),
                    )
                nc.vector.tensor_scalar_add(
                    kT[:, g * 128:(g + 1) * 128], kps[:], b_sb[:, 1:2]
                )
                vps = pkv.tile([128, 128], F32, tag="vps", name=f"vps{g}")
                for j in range(NJ):
                    c0 = g * 2048 + j * 256
                    nc.tensor.matmul(
                        vps[0:64, :], lhsT=wsl(2, j), rhs=vt_sb[:, c0:c0 + 128],
                        start=(j == 0), stop=(j == NJ - 1),
                    )
                    nc.tensor.matmul(
                        vps[64:128, :], lhsT=wsl(2, j), rhs=vt_sb[:, c0 + 128:c0 + 256],
                        start=(j == 0), stop=(j == NJ - 1),
                    )
                nc.vector.tensor_scalar_add(
                    vT[:, g * 128:(g + 1) * 128], vps[:], b_sb[:, 2:3]
                )
                for c in range(2):
                    vtr = pkv.tile([128, 64], BF16, tag="kps", name=f"vtr{g}_{c}")
                    nc.tensor.transpose(
                        vtr[:], vT[c * 64:(c + 1) * 64, g * 128:(g + 1) * 128],
                        ident_bf[c * 64:(c + 1) * 64, c * 64:(c + 1) * 64],
                    )
                    nc.vector.tensor_copy(
                        v_aug[:, (2 * g + c) * 65:(2 * g + c) * 65 + 64], vtr[:]
                    )

            def main_step(g, h, s, y_ps, m_sb):
                """Emit scores+mask+ACT for (g, s); return a closure emitting the
                y matmuls (deferred one step so the in-order PE never waits on ACT)."""
                sAB = pwork.tile([128, 1024], F32, tag="sAB", name=f"s{h}_{g}_{s}")
                qc = slice(s * 512, (s + 1) * 512)
                kc = g * 128
                # scores: 4 concurrent (K=64, M=64) tiles
                nc.tensor.matmul(
                    sAB[0:64, 0:512], lhsT=kT[0:64, kc:kc + 64],
                    rhs=qT[h][0:64, qc], start=True, stop=False,
                    skip_group_check=True,
                )
                nc.tensor.matmul(
                    sAB[64:128, 0:512], lhsT=kT[0:64, kc + 64:kc + 128],
                    rhs=qT[h][0:64, qc], start=True, stop=False,
                    skip_group_check=True,
                )
                nc.tensor.matmul(
                    sAB[0:64, 512:1024], lhsT=kT[64:128, kc:kc + 64],
                    rhs=qT[h][64:128, qc], start=True, stop=False,
                    skip_group_check=True,
                )
                nc.tensor.matmul(
                    sAB[64:128, 512:1024], lhsT=kT[64:128, kc + 64:kc + 128],
                    rhs=qT[h][64:128, qc], start=True, stop=False,
                    skip_group_check=True,
                )
                # mask add: psum += 240*m via plain fp8 identity (FWL weights)
                nc.tensor.matmul(
                    sAB[:, 0:512], lhsT=id_sb[:],
                    rhs=m_sb[:, 2 * g:2 * g + 1, s * 512:(s + 1) * 512],
                    start=False, stop=True, skip_group_check=True,
                )
                nc.tensor.matmul(
                    sAB[:, 512:1024], lhsT=id_sb[:],
                    rhs=m_sb[:, 2 * g + 1:2 * g + 2, s * 512:(s + 1) * 512],
                    start=False, stop=True, skip_group_check=True,
                )
                p = ppool.tile([128, 1024], BF16, tag="p", name=f"p{h}_{g}_{s}")
                nc.scalar.activation(p[:], sAB[:], EXP, bias=nbias[:], scale=0.125)
                if DEBUG and (g, h, s) == (0, 0, 0):
                    nc.sync.dma_start(dbg_e["d_p"][:], p[:])

                def emit_y():
                    nc.tensor.matmul(
                        y_ps[:, qc], lhsT=v_aug[:, (2 * g) * 65:(2 * g) * 65 + 65],
                        rhs=p[:, 0:512], start=(g == 0), stop=False,
                        skip_group_check=True,
                    )
                    nc.tensor.matmul(
                        y_ps[:, qc], lhsT=v_aug[:, (2 * g + 1) * 65:(2 * g + 1) * 65 + 65],
                        rhs=p[:, 512:1024], start=False, stop=(g == NG - 1),
                        skip_group_check=True,
                    )
                return emit_y

            # ---- pass 0 (q half 0) with per-group projections ----
            with nc.named_scope("qproj0"):
                q_proj(0)
            y0 = py.tile([65, 1024], F32, tag="y", name="y0")
            pend = None
            for g in range(NG):
                with nc.named_scope(f"kv{g}"):
                    kv_proj(g)
                with nc.named_scope(f"p0g{g}"):
                    for s in range(2):
                        ey = main_step(g, 0, s, y0, m0_sb)
                        if pend is not None:
                            pend()
                        pend = ey
                if g == 2:
                    # qT for half 1 while pass 0 is DMA-gated
                    with nc.named_scope("qproj1"):
                        q_proj(1)
            pend()
            ysb0 = spool.tile([65, 1024], F32, tag="ysb0")
            nc.vector.tensor_copy(ysb0[:], y0[:])
            nc.sync.dma_start(out_e[:, 0:512], ysb0[:, 0:512])
            nc.sync.dma_start(out_e[:, 512:1024], ysb0[:, 512:1024])

            # ---- pass 1 (q half 1) ----
            y1 = py.tile([65, 1024], F32, tag="y", name="y1")
            ysb1 = spool.tile([65, 1024], F32, tag="ysb1")
            pend = None
            for g in range(NG):
                with nc.named_scope(f"p1g{g}"):
                    for s in range(2):
                        ey = main_step(g, 1, s, y1, m1_sb)
                        if pend is not None:
                            pend()
                        if (g, s) == (NG - 1, 1):
                            # y region s=0 is complete: drain it now
                            nc.vector.tensor_copy(ysb1[:, 0:512], y1[:, 0:512])
                            nc.sync.dma_start(out_e[:, 1024:1536], ysb1[:, 0:512])
                        pend = ey
            pend()
            nc.vector.tensor_copy(ysb1[:, 512:1024], y1[:, 512:1024])
            nc.sync.dma_start(out_e[:, 1536:2048], ysb1[:, 512:1024])

            if DEBUG:
                nc.sync.dma_start(dbg_e["d_qT0"][:], qT[0][:])
                nc.sync.dma_start(dbg_e["d_kT"][:], kT[:])
                nc.sync.dma_start(dbg_e["d_vaug"][:], v_aug[:])

    nc.finalize()
    return nc


def _pack_x(x):
    """[2048 rows, 1024 dm] f32 -> qt layout [128, 2*8*1024] (h, j, q')."""
    t = x.T.reshape(NJ, 128, 2, 1024)          # [j, p, h, q']
    return np.ascontiguousarray(
        t.transpose(1, 2, 0, 3).reshape(128, -1)
    ).astype(ml_dtypes.bfloat16)


def _pack_kv(x):
    """[2048 keys, 1024 dm] f32 -> [128, 8*8*256] (g, j, r)."""
    t = x.T.reshape(NJ, 128, NG, 256)          # [j, p, g, r]
    return np.ascontiguousarray(
        t.transpose(1, 2, 0, 3).reshape(128, -1)
    ).astype(ml_dtypes.bfloat16)


def _pack_mask(mblk):
    """mask block [2048 q, 2048 k] int -> (m0, m1) each [128, 16, 1024] fp8.
    element (key = g*256 + j*128 + p, q = h*1024 + q') at m{h}[p, 2g+j, q']."""
    t = mblk.T.reshape(NG, 2, 128, 2, 1024)    # [g, j, p, h, q']
    t = t.transpose(2, 3, 0, 1, 4)             # [p, h, g, j, q']
    m = np.ascontiguousarray(t.reshape(128, 2, 16, 1024)).astype(ml_dtypes.float8_e4m3)
    return m[:, 0], m[:, 1]


def kernel(Q, K, V, mask, Wq, bq, Wk, bk, Wv, bv):
    global _last_results
    bf16 = ml_dtypes.bfloat16
    fp8 = ml_dtypes.float8_e4m3

    Q, K, V = (np.asarray(a, dtype=np.float32) for a in (Q, K, V))
    mask = np.asarray(mask)

    w_p = np.concatenate(
        [np.ascontiguousarray(
            W.T.reshape(NJ, 128, DK).transpose(1, 0, 2).reshape(128, NJ * DK)
         ).astype(bf16) for W in (Wq, Wk, Wv)],
        axis=1,
    )
    b_p = np.ascontiguousarray(
        np.stack([np.tile(np.asarray(b, np.float32), 2) for b in (bq, bk, bv)], axis=1)
    )
    ident = (MASK_W * np.eye(128, dtype=np.float32)).astype(fp8)

    qt_c = {(b, qh): _pack_x(Q[b, qh * SQ:(qh + 1) * SQ]) for b in range(B) for qh in range(2)}
    kt_c = {(b, kh): _pack_kv(K[b, kh * SK:(kh + 1) * SK]) for b in range(B) for kh in range(2)}
    vt_c = {(b, kh): _pack_kv(V[b, kh * SK:(kh + 1) * SK]) for b in range(B) for kh in range(2)}

    in_maps = []
    for c in range(N_CORES):
        b, r = divmod(c, 4)
        qh, kh = divmod(r, 2)
        m0, m1 = _pack_mask(mask[b, qh * SQ:(qh + 1) * SQ, kh * SK:(kh + 1) * SK])
        in_maps.append({
            "qt": qt_c[(b, qh)], "kt": kt_c[(b, kh)], "vt": vt_c[(b, kh)],
            "m0": m0, "m1": m1,
            "wqkv": w_p, "bqkv": b_p, "identdr": ident,
        })

    nc = _build()
    res = run_bass_kernel_spmd(nc, in_maps, core_ids=list(range(N_CORES)))
    _last_results = res

    out = np.empty((B, S, DK), dtype=np.float32)
    for b in range(B):
        for qh in range(2):
            yA = res.results[b * 4 + qh * 2 + 0]["out"].astype(np.float64)
            yB = res.results[b * 4 + qh * 2 + 1]["out"].astype(np.float64)
            ysum = yA + yB
            y = ysum[:DK] / ysum[DK:DK + 1]
            out[b, qh * SQ:(qh + 1) * SQ, :] = y.T.astype(np.float32)
    return out


# revision 40
# speedup vs baseline: 1.1890x; 1.1822x over previous
"""Distributed Trainium2 (8 NeuronCores) attention-head kernel, v2.

Problem: single attention head with projections.
  q = Q @ Wq.T + bq ; k = K @ Wk.T + bk ; v = V @ Wv.T + bv
  x = (q @ k.T) / 8 ; x = x*m - 1e9*(1-m) ; p = softmax(x) ; y = p @ v
Shapes: Q/K/V [2, 4096, 1024] f32, mask [2, 4096, 4096] int32 -> y [2, 4096, 64].

Sharding (8 cores): 2x2 grid per batch (flash-decoding style per the hint):
core (b, qh, kh) handles 2048 queries x 2048 keys and returns UNNORMALIZED
partial stats yT[65, 2048] = [sum_s p_s v_s ; sum_s p_s]; the host combines
the two kh partials per (b, qh): y = (yA+yB)[:64] / (yA+yB)[64].  This is the
"all-gathered softmax statistics" combine done at unshard time (collectives
on this fleet cost ~100us fixed, host combine is ~2M flops).

Device pipeline (all matmuls bf16, psum f32):
  - projections col-tiled (out width 64 -> two 64-row col strips run
    concurrently in the PE array); qT is produced duplicated on both
    partition halves, kT split even/odd chunk so scores can row-tile.
  - scores: contraction is only dk=64, so 4 (K=64, M=64) tiles run
    concurrently via tile_position row+col strips (~2x).
  - mask: folded into the scores PSUM by an fp8 DoubleRow identity matmul
    (psum += 240*m), then ACT computes p = exp(0.125*s + 30m - 30) in one
    pass - the masked softmax exactly (leak exp(-30+6) ~ 4e-11, negligible).
    No DVE/Pool elementwise mask work, mask DMA stays 1 byte/elem.
  - y: yT[65, :] += v_aug.T @ p accumulated over key chunks (v_aug has a
    ones column -> row 64 = sum p).
  - PE warmup matmuls at t=0 engage the HAM clock gate (1.2 -> 2.4 GHz).
"""

import numpy as np
import ml_dtypes

import concourse.bass as bass
import concourse.mybir as mybir
import concourse.tile as tile
from concourse import bacc
from concourse.bass_utils import run_bass_kernel_spmd
from concourse.masks import make_identity

B, S, DM, DK = 2, 4096, 1024, 64
N_CORES = 8
SQ = 2048            # queries per core
SK = 2048            # keys per core
NG = 8               # key groups per core (256 keys each)
NJ = DM // 128       # dm chunks (8)

F32 = mybir.dt.float32
BF16 = mybir.dt.bfloat16
FP8 = mybir.dt.float8e4
DR = mybir.MatmulPerfMode.DoubleRow
EXP = mybir.ActivationFunctionType.Exp

MASK_W = 240.0       # ident weight: exp(0.125*(s + 240*m) - 30) = exp(s/8 + 30m - 30)
N_WARM = 6           # PE warmup matmuls: keep PE busy until qt lands so HAM stays hot
DEBUG = False        # add intermediate dumps

_last_results = None


def _build():
    nc = bacc.Bacc(None, target_bir_lowering=False)

    qt_e = nc.declare_dram_parameter("qt", [128, 2 * NJ * 1024], BF16, isOutput=False)
    kt_e = nc.declare_dram_parameter("kt", [128, NG * NJ * 256], BF16, isOutput=False)
    vt_e = nc.declare_dram_parameter("vt", [128, NG * NJ * 256], BF16, isOutput=False)
    m0_e = nc.declare_dram_parameter("m0", [128, 16, 1024], FP8, isOutput=False)
    m1_e = nc.declare_dram_parameter("m1", [128, 16, 1024], FP8, isOutput=False)
    w_e = nc.declare_dram_parameter("wqkv", [128, 3 * NJ * DK], BF16, isOutput=False)
    b_e = nc.declare_dram_parameter("bqkv", [128, 3], F32, isOutput=False)
    id_e = nc.declare_dram_parameter("identdr", [128, 128], FP8, isOutput=False)
    out_e = nc.declare_dram_parameter("out", [65, SQ], F32, isOutput=True)
    if DEBUG:
        dbg_e = {
            "d_qT0": nc.declare_dram_parameter("d_qT0", [128, 1024], BF16, isOutput=True),
            "d_kT": nc.declare_dram_parameter("d_kT", [128, NG * 128], BF16, isOutput=True),
            "d_vaug": nc.declare_dram_parameter("d_vaug", [128, 16 * 65], BF16, isOutput=True),
            "d_p": nc.declare_dram_parameter("d_p", [128, 1024], BF16, isOutput=True),
        }

    with tile.TileContext(nc) as tc:
        with (
            tc.tile_pool(name="const", bufs=1) as cpool,
            tc.tile_pool(name="inp", bufs=1) as ipool,
            tc.tile_pool(name="work", bufs=1) as spool,
            tc.tile_pool(name="pp", bufs=3) as ppool,
            tc.tile_pool(name="ps_work", bufs=2, space="PSUM") as pwork,
            tc.tile_pool(name="ps_y", bufs=1, space="PSUM") as py,
            tc.tile_pool(name="ps_kv", bufs=1, space="PSUM") as pkv,
        ):
            # ---- constants / warmup (no DMA deps) ----
            wu = cpool.tile([128, 512], BF16, tag="wu")
            nc.vector.memset(wu[:], 0.0)
            nbias = cpool.tile([128, 1], F32, tag="nbias")
            nc.vector.memset(nbias[:], -30.0)
            act_w = spool.tile([128, 32], BF16, tag="actw")
            nc.scalar.activation(act_w[:], wu[:, 0:32], EXP, bias=nbias[:])  # pull exp tables early
            ident_bf = cpool.tile([128, 128], BF16, tag="identbf")
            make_identity(nc, ident_bf[:])

            wups = pwork.tile([128, 1024], F32, tag="sAB", name="wups")
            for i in range(N_WARM):
                nc.tensor.matmul(
                    wups[:, 0:512], lhsT=wu[:, 0:128], rhs=wu[:],
                    start=True, stop=True, skip_group_check=True,
                )

            # ---- input DMAs (issue order ~= arrival order) ----
            w_sb = cpool.tile([128, 3 * NJ * DK], BF16, tag="w")
            nc.sync.dma_start(w_sb[:], w_e[:])
            b_sb = cpool.tile([128, 3], F32, tag="b")
            nc.sync.dma_start(b_sb[:], b_e[:])
            id_sb = cpool.tile([128, 128], FP8, tag="ident")
            nc.sync.dma_start(id_sb[:], id_e[:])

            qt_sb = ipool.tile([128, 2 * NJ * 1024], BF16, tag="qt")
            kt_sb = ipool.tile([128, NG * 2048], BF16, tag="kt")
            vt_sb = ipool.tile([128, NG * 2048], BF16, tag="vt")
            m0_sb = ipool.tile([128, 16, 1024], FP8, tag="m0")
            m1_sb = ipool.tile([128, 16, 1024], FP8, tag="m1")
            nc.sync.dma_start(kt_sb[:, 0:2048], kt_e[:, 0:2048])
            nc.sync.dma_start(qt_sb[:, 0:4096], qt_e[:, 0:4096])
            nc.sync.dma_start(qt_sb[:, 4096:8192], qt_e[:, 4096:8192])
            nc.sync.dma_start(vt_sb[:, 0:2048], vt_e[:, 0:2048])
            nc.sync.dma_start(m0_sb[:, 0:2, :], m0_e[:, 0:2, :])
            for g in range(1, NG):   # per-group blocks: smoother pass-0 gating
                cs = slice(g * 2048, (g + 1) * 2048)
                nc.sync.dma_start(kt_sb[:, cs], kt_e[:, cs])
                nc.sync.dma_start(vt_sb[:, cs], vt_e[:, cs])
                nc.sync.dma_start(m0_sb[:, 2 * g:2 * g + 2, :], m0_e[:, 2 * g:2 * g + 2, :])
                if g == 2:
                    nc.sync.dma_start(qt_sb[:, 8192:16384], qt_e[:, 8192:16384])
            nc.sync.dma_start(m1_sb[:], m1_e[:])

            def wsl(which, j):
                return w_sb[:, (which * NJ + j) * DK:(which * NJ + j + 1) * DK]

            # ---- persistent work tiles ----
            qT = {}
            kT = spool.tile([128, NG * 128], BF16, tag="kT")
            vT = spool.tile([128, NG * 128], BF16, tag="vT")
            v_aug = spool.tile([128, 16 * 65], BF16, tag="vaug")
            nc.vector.memset(v_aug[:], 1.0)

            def q_proj(h):
                qps = pwork.tile([128, 1024], F32, tag="sAB", name=f"qps{h}")
                for j in range(NJ):
                    for s in range(2):
                        rhs = qt_sb[:, h * 8192 + j * 1024 + s * 512:
                                    h * 8192 + j * 1024 + (s + 1) * 512]
                        for st in range(2):
                            nc.tensor.matmul(
                                qps[st * 64:(st + 1) * 64, s * 512:(s + 1) * 512],
                                lhsT=wsl(0, j), rhs=rhs,
                                start=(j == 0), stop=(j == NJ - 1),
                            )
                qT[h] = spool.tile([128, 1024], BF16, tag=f"qT{h}", name=f"qT{h}")
                nc.vector.tensor_scalar_add(qT[h][:], qps[:], b_sb[:, 0:1])

            def k_proj(g):
                kps = pkv.tile([128, 128], F32, tag="kps", name=f"kps{g}")
                for j in range(NJ):
                    c0 = g * 2048 + j * 256
                    nc.tensor.matmul(
                        kps[0:64, :], lhsT=wsl(1, j), rhs=kt_sb[:, c0:c0 + 128],
                        start=(j == 0), stop=(j == NJ - 1),
                    )
                    nc.tensor.matmul(
                        kps[64:128, :], lhsT=wsl(1, j), rhs=kt_sb[:, c0 + 128:c0 + 256],
                        start=(j == 0), stop=(j == NJ - 1),
                    )
                nc.vector.tensor_scalar_add(
                    kT[:, g * 128:(g + 1) * 128], kps[:], b_sb[:, 1:2]
                )

            def v_proj(g):
                vps = pkv.tile([128, 128], F32, tag="vps", name=f"vps{g}")
                for j in range(NJ):
                    c0 = g * 2048 + j * 256
                    nc.tensor.matmul(
                        vps[0:64, :], lhsT=wsl(2, j), rhs=vt_sb[:, c0:c0 + 128],
                        start=(j == 0), stop=(j == NJ - 1),
                    )
                    nc.tensor.matmul(
                        vps[64:128, :], lhsT=wsl(2, j), rhs=vt_sb[:, c0 + 128:c0 + 256],
                        start=(j == 0), stop=(j == NJ - 1),
                    )
                nc.vector.tensor_scalar_add(
                    vT[:, g * 128:(g + 1) * 128], vps[:], b_sb[:, 2:3]
                )
                for c in range(2):
                    vtr = pkv.tile([128, 64], BF16, tag="kps", name=f"vtr{g}_{c}")
                    nc.tensor.transpose(
                        vtr[:], vT[c * 64:(c + 1) * 64, g * 128:(g + 1) * 128],
                        ident_bf[c * 64:(c + 1) * 64, c * 64:(c + 1) * 64],
                    )
                    nc.vector.tensor_copy(
                        v_aug[:, (2 * g + c) * 65:(2 * g + c) * 65 + 64], vtr[:]
                    )

            def main_step(g, h, s, y_ps, m_sb):
                """Emit scores+mask+ACT for (g, s); return a closure emitting the
                y matmuls (deferred one step so the in-order PE never waits on ACT)."""
                sAB = pwork.tile([128, 1024], F32, tag="sAB", name=f"s{h}_{g}_{s}")
                qc = slice(s * 512, (s + 1) * 512)
                kc = g * 128
                # scores: 4 concurrent (K=64, M=64) tiles
                nc.tensor.matmul(
                    sAB[0:64, 0:512], lhsT=kT[0:64, kc:kc + 64],
                    rhs=qT[h][0:64, qc], start=True, stop=False,
                    skip_group_check=True,
                )
                nc.tensor.matmul(
                    sAB[64:128, 0:512], lhsT=kT[0:64, kc + 64:kc + 128],
                    rhs=qT[h][0:64, qc], start=True, stop=False,
                    skip_group_check=True,
                )
                nc.tensor.matmul(
                    sAB[0:64, 512:1024], lhsT=kT[64:128, kc:kc + 64],
                    rhs=qT[h][64:128, qc], start=True, stop=False,
                    skip_group_check=True,
                )
                nc.tensor.matmul(
                    sAB[64:128, 512:1024], lhsT=kT[64:128, kc + 64:kc + 128],
                    rhs=qT[h][64:128, qc], start=True, stop=False,
                    skip_group_check=True,
                )
                # mask add: psum += 240*m via plain fp8 identity (FWL weights)
                nc.tensor.matmul(
                    sAB[:, 0:512], lhsT=id_sb[:],
                    rhs=m_sb[:, 2 * g:2 * g + 1, s * 512:(s + 1) * 512],
                    start=False, stop=True, skip_group_check=True,
                )
                nc.tensor.matmul(
                    sAB[:, 512:1024], lhsT=id_sb[:],
                    rhs=m_sb[:, 2 * g + 1:2 * g + 2, s * 512:(s + 1) * 512],
                    start=False, stop=True, skip_group_check=True,
                )
                p = ppool.tile([128, 1024], BF16, tag="p", name=f"p{h}_{g}_{s}")
                nc.scalar.activation(p[:], sAB[:], EXP, bias=nbias[:], scale=0.125)
                if DEBUG and (g, h, s) == (0, 0, 0):
                    nc.sync.dma_start(dbg_e["d_p"][:], p[:])

                def emit_y():
                    nc.tensor.matmul(
                        y_ps[:, qc], lhsT=v_aug[:, (2 * g) * 65:(2 * g) * 65 + 65],
                        rhs=p[:, 0:512], start=(g == 0), stop=False,
                        skip_group_check=True,
                    )
                    nc.tensor.matmul(
                        y_ps[:, qc], lhsT=v_aug[:, (2 * g + 1) * 65:(2 * g + 1) * 65 + 65],
                        rhs=p[:, 512:1024], start=False, stop=(g == NG - 1),
                        skip_group_check=True,
                    )
                return emit_y

            # ---- pass 0 (q half 0) with per-group projections ----
            # k-proj(g0) first: its kt block lands before qt does
            with nc.named_scope("kp0"):
                k_proj(0)
            with nc.named_scope("qproj0"):
                q_proj(0)
            y0 = py.tile([65, 1024], F32, tag="y", name="y0")
            pend = None
            for g in range(NG):
                with nc.named_scope(f"kv{g}"):
                    if g > 0:
                        k_proj(g)
                    v_proj(g)
                with nc.named_scope(f"p0g{g}"):
                    for s in range(2):
                        ey = main_step(g, 0, s, y0, m0_sb)
                        if pend is not None:
                            pend()
                        pend = ey
                if g == 2:
                    # qT for half 1 while pass 0 is DMA-gated
                    with nc.named_scope("qproj1"):
                        q_proj(1)
            pend()
            ysb0 = spool.tile([65, 1024], F32, tag="ysb0")
            nc.vector.tensor_copy(ysb0[:], y0[:])
            nc.sync.dma_start(out_e[:, 0:512], ysb0[:, 0:512])
            nc.sync.dma_start(out_e[:, 512:1024], ysb0[:, 512:1024])

            # ---- pass 1 (q half 1) ----
            y1 = py.tile([65, 1024], F32, tag="y", name="y1")
            ysb1 = spool.tile([65, 1024], F32, tag="ysb1")
            pend = None
            for g in range(NG):
                with nc.named_scope(f"p1g{g}"):
                    for s in range(2):
                        ey = main_step(g, 1, s, y1, m1_sb)
                        if pend is not None:
                            pend()
                        if (g, s) == (NG - 1, 1):
                            # y region s=0 is complete: drain it now
                            nc.vector.tensor_copy(ysb1[:, 0:512], y1[:, 0:512])
                            nc.sync.dma_start(out_e[:, 1024:1536], ysb1[:, 0:512])
                        pend = ey
            pend()
            nc.vector.tensor_copy(ysb1[:, 512:1024], y1[:, 512:1024])
            nc.sync.dma_start(out_e[:, 1536:2048], ysb1[:, 512:1024])

            if DEBUG:
                nc.sync.dma_start(dbg_e["d_qT0"][:], qT[0][:])
                nc.sync.dma_start(dbg_e["d_kT"][:], kT[:])
                nc.sync.dma_start(dbg_e["d_vaug"][:], v_aug[:])

    nc.finalize()
    return nc


def _pack_x(x):
    """[2048 rows, 1024 dm] f32 -> qt layout [128, 2*8*1024] (h, j, q')."""
    t = x.T.reshape(NJ, 128, 2, 1024)          # [j, p, h, q']
    return np.ascontiguousarray(
        t.transpose(1, 2, 0, 3).reshape(128, -1)
    ).astype(ml_dtypes.bfloat16)


def _pack_kv(x):
    """[2048 keys, 1024 dm] f32 -> [128, 8*8*256] (g, j, r)."""
    t = x.T.reshape(NJ, 128, NG, 256)          # [j, p, g, r]
    return np.ascontiguousarray(
        t.transpose(1, 2, 0, 3).reshape(128, -1)
    ).astype(ml_dtypes.bfloat16)


def _pack_mask(mblk):
    """mask block [2048 q, 2048 k] int -> (m0, m1) each [128, 16, 1024] fp8.
    element (key = g*256 + j*128 + p, q = h*1024 + q') at m{h}[p, 2g+j, q']."""
    t = mblk.T.reshape(NG, 2, 128, 2, 1024)    # [g, j, p, h, q']
    t = t.transpose(2, 3, 0, 1, 4)             # [p, h, g, j, q']
    m = np.ascontiguousarray(t.reshape(128, 2, 16, 1024)).astype(ml_dtypes.float8_e4m3)
    return m[:, 0], m[:, 1]


def kernel(Q, K, V, mask, Wq, bq, Wk, bk, Wv, bv):
    global _last_results
    bf16 = ml_dtypes.bfloat16
    fp8 = ml_dtypes.float8_e4m3

    Q, K, V = (np.asarray(a, dtype=np.float32) for a in (Q, K, V))
    mask = np.asarray(mask)

    w_p = np.concatenate(
        [np.ascontiguousarray(
            W.T.reshape(NJ, 128, DK).transpose(1, 0, 2).reshape(128, NJ * DK)
         ).astype(bf16) for W in (Wq, Wk, Wv)],
        axis=1,
    )
    b_p = np.ascontiguousarray(
        np.stack([np.tile(np.asarray(b, np.float32), 2) for b in (bq, bk, bv)], axis=1)
    )
    ident = (MASK_W * np.eye(128, dtype=np.float32)).astype(fp8)

    qt_c = {(b, qh): _pack_x(Q[b, qh * SQ:(qh + 1) * SQ]) for b in range(B) for qh in range(2)}
    kt_c = {(b, kh): _pack_kv(K[b, kh * SK:(kh + 1) * SK]) for b in range(B) for kh in range(2)}
    vt_c = {(b, kh): _pack_kv(V[b, kh * SK:(kh + 1) * SK]) for b in range(B) for kh in range(2)}

    in_maps = []
    for c in range(N_CORES):
        b, r = divmod(c, 4)
        qh, kh = divmod(r, 2)
        m0, m1 = _pack_mask(mask[b, qh * SQ:(qh + 1) * SQ, kh * SK:(kh + 1) * SK])
        in_maps.append({
            "qt": qt_c[(b, qh)], "kt": kt_c[(b, kh)], "vt": vt_c[(b, kh)],
            "m0": m0, "m1": m1,
            "wqkv": w_p, "bqkv": b_p, "identdr": ident,
        })

    nc = _build()
    res = run_bass_kernel_spmd(nc, in_maps, core_ids=list(range(N_CORES)))
    _last_results = res

    out = np.empty((B, S, DK), dtype=np.float32)
    for b in range(B):
        for qh in range(2):
            yA = res.results[b * 4 + qh * 2 + 0]["out"].astype(np.float64)
            yB = res.results[b * 4 + qh * 2 + 1]["out"].astype(np.float64)
            ysum = yA + yB
            y = ysum[:DK] / ysum[DK:DK + 1]
            out[b, qh * SQ:(qh + 1) * SQ, :] = y.T.astype(np.float32)
    return out
